# revision 41
# baseline (speedup 1.0000x reference)
"""Trainium2 Bass kernel for nn_CARLA_55430847922453.

Reference computation (per image, single step):
  x    = min(obs + (noise < hallucinogen), 1.0)          # hallucinate
  grid = 9 circular shifts of x (3x3 one-hot stencil)    # perceive
  h    = relu(w1 @ grid)                                 # 1x1 conv 9->32
  z    = w2 @ h + b2                                     # 1x1 conv 32->1
  out  = (sigmoid(z) > 0.5)  ==  (z > 0)                 # threshold
  out  = center crop 64x64 of the 256x256 grid

Key structural facts exploited:
  * Only the central 64x64 output crop is returned, so only a 66x66 input
    crop (rows/cols 95..160) feeds the result; no circular wrap occurs
    there.  The kernel reads just that window from DRAM.
  * obs in [0,1): min(obs + ind, 1) == max(obs, ind), ind = (noise < p).
  * sigmoid(z) > 0.5  <=>  z > 0, so no sigmoid is needed.
  * |w2_o| folds into w1 row o (relu(s*a) = s*relu(a), s>=0), leaving a
    +/-1 (sign) second layer.
  * bf16 matmul inputs are safe: decision margin |z| ~ 0.5 >> bf16 error.

Sharding: pure data parallel, 4 images per core across 8 cores.

Per-core dataflow:
  crops (2 DMAs) -> select (2 DVE ops) -> bf16 scratch in DRAM ->
  band tile [12, 4224] (1 contiguous DMA) -> pixel-form G [36, 4096]
  (shifted pixel-extract copies, split across both DMA rings and in
  column halves so the matmul loop starts after the first half) ->
  8x [ mm1 (K=36 blockdiag -> M=128) -> relu copy (ACT/DVE split) -> mm2
       (K=128 -> M=32 sign blockdiag, PE col-tile per chunk so z lands on
       16 PSUM partitions) ] ->
  fused bias+threshold DVE ops (the first z block overlaps the loop's
  second half) -> output DMAs split across both rings.

All weight/bias/threshold structures are built ON-CHIP (PE outer-product
broadcasts, a PE transpose against diag(|w2|), selector matrices and
blockdiag masks from gpsimd affine_select): DRAM parameter traffic is one
32-descriptor w1 load plus three 1-element loads.  This matters because
sub-32B HBM accesses cost ~50ns each (read-modify-write) and were the
dominant term in earlier revisions.

Written in raw Bass (explicit semaphores): the Tile layer's generated
sync exceeds this walrus build's per-instruction sync-slot limits (~2
waits per compute instruction).  Other ISA constraints honored here:
DMA access patterns balance to <= 3 dims, engine access patterns need
32-aligned partition starts, the PE stationary operand allows only one
free dim, and semaphore waits must target race-free (batch-sum) values.
"""

from contextlib import ExitStack

import numpy as np

import concourse.bass as bass
from concourse import mybir

F32 = mybir.dt.float32
BF16 = mybir.dt.bfloat16
I32 = mybir.dt.int32
AF = mybir.ActivationFunctionType
ALU = mybir.AluOpType

B, H, W = 32, 256, 256
N_CORES = 8
BL = B // N_CORES          # images per core
OUT = 64                   # output crop side
C0 = (H - OUT) // 2 - 1    # 95: first input row/col needed (1-pixel halo)
CS = OUT + 2               # 66: crop side with halo
HID = 32                   # hidden channels
NPIX = OUT * OUT           # pixels per image
CHUNK = 512
NCHUNK = NPIX // CHUNK     # 8
N_WARM2 = 0                # PE clock is throttle-capped; warmups useless
KDIM = 36                  # mm1 contraction: 3 c's x (4 images x 3 r's)
MDIM = BL * HID            # 128
BAND = OUT * CS            # 4224: one (image, r) band, 66-wide rows
N_WARM = 10                # PE warmup matmuls (HAM clock ungating)
WARM_N = 256
NHBUF = 4                  # h_bf rotation depth
NPBUF = 3                  # h_ps psum rotation depth


def _ap(t, offset, dims):
    """Raw access pattern on tensor/AP t's underlying tensor."""
    if not isinstance(t, bass.AP):
        t = t[:]
    return bass.AP(tensor=t.tensor, offset=t.offset + offset, ap=list(dims))


def build_program():
    nc = bass.Bass()

    obs = nc.declare_dram_parameter("obs", [BL, 1, H, W], F32, isOutput=False)
    noise = nc.declare_dram_parameter("noise", [BL, 1, H, W], F32, isOutput=False)
    w1 = nc.declare_dram_parameter("w1", [HID, 9], F32, isOutput=False)
    w2 = nc.declare_dram_parameter("w2", [1, HID], F32, isOutput=False)
    b2 = nc.declare_dram_parameter("b2", [1], F32, isOutput=False)
    hal = nc.declare_dram_parameter("hallucinogen", [1], F32, isOutput=False)
    act = nc.declare_dram_parameter("act", [BL, 1, OUT, OUT], F32, isOutput=True)

    scratch = nc.dram_tensor("scratch", [12, BAND], BF16)  # band-form
    w1_scr = nc.dram_tensor("w1_scr", [9, HID], BF16)  # [kappa, o]

    ctx = ExitStack()
    with ctx:
        ctx.enter_context(nc.allow_non_contiguous_dma(
            reason="tiny parameter-prep transposes/broadcasts"))
        names = [0]

        def sb(shape, dt):
            names[0] += 1
            return ctx.enter_context(
                nc.sbuf_tensor(f"sb{names[0]}", shape, dt))

        def ps(shape):
            names[0] += 1
            return ctx.enter_context(
                nc.psum_tensor(f"ps{names[0]}", shape, F32))

        # SBUF tensors
        obs_c = sb([CS, BL * CS], F32)
        noise_c = sb([CS, BL * CS], F32)
        ind = sb([CS, BL * CS], F32)
        x_bf = sb([CS, BL * CS], BF16)
        band = sb([12, BAND], BF16)
        gt = sb([KDIM, NPIX], BF16)
        junk_bf = sb([KDIM, WARM_N], BF16)
        ones_f32 = sb([1, 128], F32)
        ones_bf = sb([1, 128], BF16)
        ones32 = sb([32, 32], BF16)
        i32 = sb([32, 32], BF16)
        ones9 = sb([9, KDIM], BF16)
        sel9 = sb([9, KDIM], BF16)
        w1t_sb = sb([9, HID], BF16)
        w1_sb = sb([HID, 9], F32)
        w1_bf = sb([HID, 9], BF16)
        w2row = sb([1, HID], F32)
        w2abs_row = sb([1, HID], BF16)
        sgn_row = sb([1, HID], BF16)
        b2row = sb([1, 1], F32)
        p_row = sb([1, 1], F32)
        diag32 = sb([32, 32], BF16)
        w1rep = sb([KDIM, MDIM], BF16)
        t_band = [sb([KDIM, MDIM], BF16) for _ in range(3)]
        b_band = [sb([KDIM, MDIM], BF16) for _ in range(3)]
        w1blk = sb([KDIM, MDIM], BF16)
        diagmask = sb([MDIM, HID], BF16)
        w2blk = sb([MDIM, HID], BF16)
        h_bf = [sb([MDIM, CHUNK], BF16) for _ in range(NHBUF)]
        out_sb = sb([128, 2 * CHUNK], F32)

        # PSUM: warm(1 bank) + misc(1) + 3x h(3) + z(2) = 7 banks
        warm_ps = ps([1, WARM_N])
        misc_ps = ps([128, CHUNK])
        p_ps = misc_ps[0:CS, 0:1]
        b2_ps = misc_ps[0:128, 1:2]
        w2abs_ps = misc_ps[0:HID, 2:34]
        sgn_ps = misc_ps[0:MDIM, 34:66]
        w1x_ps = misc_ps[0:KDIM, 66:98]
        w1t_ps = misc_ps[0:9, 98:130]
        h_ps = [ps([MDIM, CHUNK]) for _ in range(NPBUF)]
        z_ps = ps([128, 2 * CHUNK])

        sA = ctx.enter_context(nc.semaphore("s_dma_sync"))   # sync-ring DMAs
        sB = ctx.enter_context(nc.semaphore("s_dma_act"))    # act-ring DMAs
        s_dve = ctx.enter_context(nc.semaphore("s_dve"))
        s_act = ctx.enter_context(nc.semaphore("s_act"))
        s_pool = ctx.enter_context(nc.semaphore("s_pool"))
        s_pe = ctx.enter_context(nc.semaphore("s_pe"))

        blk = ctx.enter_context(nc.Block())

        # relu engine split: even chunks on ACT, odd on DVE
        relu_on_act = [i % 2 == 0 for i in range(NCHUNK)]
        # --- engine-sem checkpoints (instruction counts per engine) ---
        DVE_XBF = 8      # junk, 4x ones, w1_bf, ind, x_bf
        DVE_W1REP = 14   # + diag32, w1t copy, 4x w1rep
        DVE_PREP = 22    # + 2 band adds, 6 diagmask/w2blk ops
        THRESH0 = DVE_PREP + 2 + 1   # 2 DVE relus (chunks 1,3) + thresh a=0
        THRESH = DVE_PREP + NCHUNK // 2 + 2   # all relus + both thresholds
        ACT_PREP = 2     # Abs, Sign
        PE_BASE = N_WARM + 6 + N_WARM2   # warmups + 6 prep mms + warmups2
        P_MMP = N_WARM + 1
        P_MMA = N_WARM + 3
        P_MMS = N_WARM + 4
        P_MMT = N_WARM + 5
        P_MMX = N_WARM + 6
        GH1_A = 6 * 16       # sA: crops, band writes 0-1, first-half c=0,1
        GH2_A = 8 * 16       # sA: + second-half c=0,1
        GH1_B = 6 * 16       # sB: params, band write 2, first-half c=2
        GH2_B = 7 * 16       # sB: + second-half c=2
        P_GATE = 2 * 16      # sB: p and b2 loads done
        WARM2_GATE = 4 * 16  # sA: band writes done (stable sem value)
        PARAMS = 4 * 16    # sB: w1, w2, b2, hal loads

        def dve_relu_count(i):
            n = DVE_PREP + sum(1 for j in range(i + 1) if not relu_on_act[j])
            return n + 1 if i >= 4 else n   # thresh a=0 interleaves after chunk 3

        def act_relu_count(i):
            return ACT_PREP + sum(1 for j in range(i + 1) if relu_on_act[j])

        def pe_mm1_count(i):
            # PE order: prep, mm1_0, mm1_1, [mm2_0, mm1_2], ..., mm2_6, mm2_7
            return PE_BASE + 1 + i if i < 2 else PE_BASE + 2 * i

        def pe_mm2_count(i):
            if i < NCHUNK - 2:
                return PE_BASE + 3 + 2 * i
            return PE_BASE + 2 * NCHUNK - (NCHUNK - 1 - i)

        @blk.sync
        def _(sync):
            cum = [0]

            def dma(out, in_):
                sync.dma_start(out=out, in_=in_).then_inc(sA, 16)
                cum[0] += 16

            crop_src = [[W, CS], [H * W, BL], [1, CS]]
            dma(obs_c[:], _ap(obs, C0 * W + C0, crop_src))
            dma(noise_c[:], _ap(noise, C0 * W + C0, crop_src))
            # band-form scratch: row 3g+r = crop_g rows r..r+64, 66-wide;
            # one write per r (r=0,1 here, r=2 on the ACT ring in parallel)
            sync.wait_ge(s_dve, DVE_XBF)
            for r in range(2):
                dma(_ap(scratch, r * BAND, [[CS, OUT], [3 * BAND, BL], [1, CS]]),
                    x_bf[r:r + OUT, :])
            # pixel-extract copies straight from DRAM scratch, in column
            # quarters so the matmul loop starts after the first quarter
            for q in range(4):
                sync.wait_ge(sA, cum[0])
                if q == 0:
                    sync.wait_ge(sB, 5 * 16)   # band write r=2 done
                for c in range(2):
                    dma(gt[12 * c:12 * (c + 1), 1024 * q:1024 * (q + 1)],
                        _ap(scratch, c + 16 * q * CS,
                            [[BAND, 12], [CS, 16], [1, OUT]]))
            # output (first two column groups; other two on the ACT ring)
            sync.wait_ge(sA, cum[0])
            sync.wait_ge(s_dve, THRESH0)
            for c in range(2):
                dma(_ap(act, c * CHUNK, [[NPIX, BL], [1, CHUNK]]),
                    out_sb[32 * c:32 * c + BL, 0:CHUNK])
            sync.wait_ge(s_dve, THRESH)
            for c in range(2):
                dma(_ap(act, c * CHUNK + 4 * CHUNK, [[NPIX, BL], [1, CHUNK]]),
                    out_sb[32 * c:32 * c + BL, CHUNK:])

        @blk.scalar
        def _(scalar):
            def dma(out, in_):
                scalar.dma_start(out=out, in_=in_).then_inc(sB, 16)

            dma(p_row[:], _ap(hal, 0, [[1, 1], [1, 1]]))
            dma(b2row[:], _ap(b2, 0, [[1, 1], [1, 1]]))
            scalar.wait_ge(sB, P_GATE)
            dma(w1_sb[:], w1[:, :])
            dma(w2row[:], w2[:, :])
            scalar.wait_ge(sB, PARAMS)
            nc.scalar.activation(w2abs_row[:], w2row[:], AF.Abs).then_inc(s_act, 1)
            nc.scalar.activation(sgn_row[:], w2row[:], AF.Sign).then_inc(s_act, 1)
            # band write r=2, then the c=2 pixel copies on this ring
            scalar.wait_ge(s_dve, DVE_XBF)
            dma(_ap(scratch, 2 * BAND, [[CS, OUT], [3 * BAND, BL], [1, CS]]),
                x_bf[2:2 + OUT, :])
            scalar.wait_ge(sB, 5 * 16)
            scalar.wait_ge(sA, 4 * 16)   # band writes r=0,1 done
            for q in range(4):
                if q:
                    scalar.wait_ge(sB, (5 + q) * 16)
                dma(gt[24:36, 1024 * q:1024 * (q + 1)],
                    _ap(scratch, 2 + 16 * q * CS,
                        [[BAND, 12], [CS, 16], [1, OUT]]))
            # ACT relu copies
            for i in range(NCHUNK):
                if relu_on_act[i]:
                    scalar.wait_ge(s_pe, pe_mm1_count(i))
                    if i >= NHBUF:  # WAR on h_bf slot
                        scalar.wait_ge(s_pe, pe_mm2_count(i - NHBUF))
                    nc.scalar.activation(h_bf[i % NHBUF][:], h_ps[i % NPBUF][:],
                                         AF.Relu).then_inc(s_act, 1)
            # output (remaining two column groups)
            scalar.wait_ge(sB, 9 * 16)
            scalar.wait_ge(s_dve, THRESH0)
            for c in range(2, 4):
                dma(_ap(act, c * CHUNK, [[NPIX, BL], [1, CHUNK]]),
                    out_sb[32 * c:32 * c + BL, 0:CHUNK])
            scalar.wait_ge(s_dve, THRESH)
            for c in range(2, 4):
                dma(_ap(act, c * CHUNK + 4 * CHUNK, [[NPIX, BL], [1, CHUNK]]),
                    out_sb[32 * c:32 * c + BL, CHUNK:])

        @blk.gpsimd
        def _(gpsimd):
            pn = [0]

            def inc(instr):
                instr.then_inc(s_pool, 1)
                pn[0] += 1

            # identity I32 via two is_ge selects (iota v = p - f)
            gpsimd.wait_ge(s_dve, 4)   # ones32
            inc(nc.gpsimd.affine_select(
                i32[:], ones32[:], pattern=[[-1, 32]], compare_op=ALU.is_ge,
                fill=0.0, base=0, channel_multiplier=1))
            gpsimd.wait_ge(s_pool, pn[0])
            inc(nc.gpsimd.affine_select(
                i32[:], i32[:], pattern=[[1, 32]], compare_op=ALU.is_ge,
                fill=0.0, base=0, channel_multiplier=-1))
            # selector sel9[kappa, 12c+3g+r] = [kappa == 3r+c]
            gpsimd.wait_ge(s_dve, 5)   # ones9
            selpat = [[1, 3], [0, BL], [3, 3]]
            inc(nc.gpsimd.affine_select(
                sel9[:], ones9[:], pattern=selpat, compare_op=ALU.is_ge,
                fill=0.0, base=0, channel_multiplier=-1))
            gpsimd.wait_ge(s_pool, pn[0])
            inc(nc.gpsimd.affine_select(
                sel9[:], sel9[:], pattern=[[-1, 3], [0, BL], [-3, 3]],
                compare_op=ALU.is_ge, fill=0.0, base=0, channel_multiplier=1))
            # blockdiag bands: keep w1rep where 0 <= k - 3g' - 12c <= 2
            gpsimd.wait_ge(s_dve, DVE_W1REP)
            pat = [[-3, BL], [0, HID]]
            for c in range(3):
                inc(nc.gpsimd.affine_select(
                    t_band[c][:], w1rep[:], pattern=pat, compare_op=ALU.is_ge,
                    fill=0.0, base=-12 * c, channel_multiplier=1))
                gpsimd.wait_ge(s_pool, pn[0])
                inc(nc.gpsimd.affine_select(
                    b_band[c][:], t_band[c][:],
                    pattern=[[3, BL], [0, HID]], compare_op=ALU.is_ge,
                    fill=0.0, base=12 * c + 2, channel_multiplier=-1))

        POOL_I32 = 2
        POOL_SEL = 4
        POOL_BANDS = 10

        @blk.vector
        def _(vector):
            dv = [0]

            def inc(instr):
                instr.then_inc(s_dve, 1)
                dv[0] += 1

            def selfwait():
                vector.wait_ge(s_dve, dv[0])

            # 1-4: constants
            inc(nc.vector.memset(junk_bf[:], 0.5))
            inc(nc.vector.memset(ones_f32[:], 1.0))
            inc(nc.vector.memset(ones_bf[:], 1.0))
            inc(nc.vector.memset(ones32[:], 1.0))
            inc(nc.vector.memset(ones9[:], 1.0))
            # w1 -> bf16
            vector.wait_ge(sB, PARAMS)
            inc(nc.vector.tensor_copy(w1_bf[:], w1_sb[:]))
            # 6-7: hallucinate select (p broadcast sits in PSUM via PE)
            vector.wait_ge(sA, 32)
            vector.wait_ge(s_pe, P_MMP)
            inc(nc.vector.tensor_scalar(ind[:], noise_c[:], p_ps, None,
                                        ALU.is_lt))
            selfwait()
            inc(nc.vector.tensor_max(x_bf[:], ind[:], obs_c[:]))
            # 8: diag(|w2|)
            vector.wait_ge(s_pool, POOL_I32)
            vector.wait_ge(s_pe, P_MMA)
            inc(nc.vector.tensor_mul(diag32[:], i32[:], w2abs_ps))
            # w1t (transposed, |w2|-scaled) psum -> sbuf
            vector.wait_ge(s_pe, P_MMT)
            inc(nc.vector.tensor_copy(w1t_sb[:], w1t_ps))
            # w1rep from the PE-built staging
            vector.wait_ge(s_pe, P_MMX)
            for g in range(BL):
                inc(nc.vector.tensor_copy(w1rep[:, HID * g:HID * (g + 1)],
                                          w1x_ps))
            # 13-14: w1blk = sum of the 3 disjoint bands
            vector.wait_ge(s_pool, POOL_BANDS)
            inc(nc.vector.tensor_add(w1blk[:], b_band[0][:], b_band[1][:]))
            selfwait()
            inc(nc.vector.tensor_add(w1blk[:], w1blk[:], b_band[2][:]))
            # 15-20: w2blk = diagmask * sgn broadcast
            inc(nc.vector.memset(diagmask[:], 0.0))
            selfwait()
            for g in range(BL):
                inc(nc.vector.memset(diagmask[HID * g:HID * (g + 1), g:g + 1],
                                     1.0))
            vector.wait_ge(s_pe, P_MMS)
            selfwait()
            inc(nc.vector.tensor_mul(w2blk[:], diagmask[:], sgn_ps))
            # == DVE_PREP ops ==
            # DVE relu copies, with the a=0 threshold interleaved after
            # chunk 3 (its z block is complete then, overlapping chunks 4-7)
            for i in range(NCHUNK):
                if not relu_on_act[i]:
                    vector.wait_ge(s_pe, pe_mm1_count(i))
                    if i >= NHBUF:
                        vector.wait_ge(s_pe, pe_mm2_count(i - NHBUF))
                    inc(nc.vector.tensor_scalar_max(h_bf[i % NHBUF][:],
                                                    h_ps[i % NPBUF][:], 0.0))
                if i == 3:
                    vector.wait_ge(s_pe, pe_mm2_count(3))
                    inc(nc.vector.tensor_scalar(
                        out_sb[:, 0:CHUNK], z_ps[:, 0:CHUNK],
                        b2_ps, 0.0, ALU.add, ALU.is_gt))
            vector.wait_ge(s_pe, PE_BASE + 2 * NCHUNK)
            inc(nc.vector.tensor_scalar(out_sb[:, CHUNK:], z_ps[:, CHUNK:],
                                        b2_ps, 0.0, ALU.add, ALU.is_gt))

        @blk.tensor
        def _(tensor):
            # warmup (HAM): junk matmuls
            tensor.wait_ge(s_dve, 1)
            for _ in range(N_WARM):
                nc.tensor.matmul(warm_ps[:], lhsT=junk_bf[:, 0:1],
                                 rhs=junk_bf[:], start=True,
                                 stop=True).then_inc(s_pe, 1)
            # parameter broadcasts / transposes, all on-chip
            tensor.wait_ge(sB, P_GATE)
            tensor.wait_ge(s_dve, 4)
            nc.tensor.matmul(p_ps, lhsT=ones_f32[:, 0:CS], rhs=p_row[:],
                             start=True, stop=True).then_inc(s_pe, 1)
            nc.tensor.matmul(b2_ps, lhsT=ones_f32[:], rhs=b2row[:],
                             start=True, stop=True).then_inc(s_pe, 1)
            tensor.wait_ge(s_act, 2)
            nc.tensor.matmul(w2abs_ps, lhsT=w2abs_row[:],
                             rhs=ones_bf[:, 0:HID],
                             start=True, stop=True).then_inc(s_pe, 1)
            nc.tensor.matmul(sgn_ps, lhsT=ones_bf[:], rhs=sgn_row[:],
                             start=True, stop=True).then_inc(s_pe, 1)
            # w1t [9, 32] = w1_bf.T @ diag(|w2|)  (transpose + scale)
            tensor.wait_ge(s_dve, 9)   # diag32
            nc.tensor.matmul(w1t_ps, lhsT=w1_bf[:], rhs=diag32[:],
                             start=True, stop=True).then_inc(s_pe, 1)
            # staging [36, 32] = sel9.T @ w1t  (row replication/permutation)
            tensor.wait_ge(s_dve, 10)  # w1t_sb
            tensor.wait_ge(s_pool, POOL_SEL)
            nc.tensor.matmul(w1x_ps, lhsT=sel9[:], rhs=w1t_sb[:],
                             start=True, stop=True).then_inc(s_pe, 1)

            # second warmup batch: keeps the HAM busy-window alive while
            # the G copies finish, so the loop runs at 2.4 GHz
            tensor.wait_ge(sA, WARM2_GATE)
            for _ in range(N_WARM2):
                nc.tensor.matmul(warm_ps[:], lhsT=junk_bf[:, 0:1],
                                 rhs=junk_bf[:], start=True,
                                 stop=True).then_inc(s_pe, 1)

            tensor.wait_ge(s_dve, DVE_PREP)

            def mm1(i):
                if i % 2 == 0:   # G quarter q = i//2 needed from chunk 2q on
                    q = i // 2
                    tensor.wait_ge(sA, 64 + 32 * (q + 1))
                    tensor.wait_ge(sB, 80 + 16 * (q + 1))
                if i >= NPBUF:  # WAR: h_ps[i%NPBUF] drained by its relu
                    j = i - NPBUF
                    if relu_on_act[j]:
                        tensor.wait_ge(s_act, act_relu_count(j))
                    else:
                        tensor.wait_ge(s_dve, dve_relu_count(j))
                nc.tensor.matmul(h_ps[i % NPBUF][:], lhsT=w1blk[:],
                                 rhs=gt[:, i * CHUNK:(i + 1) * CHUNK],
                                 start=True, stop=True).then_inc(s_pe, 1)

            def mm2(i):
                if relu_on_act[i]:
                    tensor.wait_ge(s_act, act_relu_count(i))
                else:
                    tensor.wait_ge(s_dve, dve_relu_count(i))
                c, a = i % 4, i // 4
                nc.tensor.matmul(
                    z_ps[32 * c:32 * (c + 1), CHUNK * a:CHUNK * a + CHUNK],
                    lhsT=w2blk[:], rhs=h_bf[i % NHBUF][:],
                    start=True, stop=True,
                    tile_position=(0, 32 * c)).then_inc(s_pe, 1)

            mm1(0)
            mm1(1)
            for i in range(NCHUNK - 2):
                mm2(i)
                mm1(i + 2)
            mm2(NCHUNK - 2)
            mm2(NCHUNK - 1)

    return nc


_CACHED_NC = None


def kernel(**inputs):
    global _CACHED_NC
    if _CACHED_NC is None:
        _CACHED_NC = build_program()
    nc = _CACHED_NC

    obs = np.ascontiguousarray(inputs["obs"], dtype=np.float32)
    noise = np.ascontiguousarray(inputs["noise"], dtype=np.float32)
    w1 = np.ascontiguousarray(inputs["w1"], dtype=np.float32)
    w2 = np.ascontiguousarray(inputs["w2"], dtype=np.float32)
    b2 = np.ascontiguousarray(inputs["b2"], dtype=np.float32)
    hal = np.ascontiguousarray(inputs["hallucinogen"], dtype=np.float32)

    in_maps = []
    for c in range(N_CORES):
        sl = slice(c * BL, (c + 1) * BL)
        in_maps.append({
            "obs": np.ascontiguousarray(obs[sl]),
            "noise": np.ascontiguousarray(noise[sl]),
            "w1": w1, "w2": w2, "b2": b2, "hallucinogen": hal,
        })

    from concourse.bass_utils import run_bass_kernel_spmd
    res = run_bass_kernel_spmd(nc, in_maps, list(range(N_CORES)))
    kernel.last_results = res
    out = np.concatenate([res.results[c]["act"] for c in range(N_CORES)], axis=0)
    return out.astype(np.float32)


# revision 43
# speedup vs baseline: 1.1400x; 1.1400x over previous
"""Trainium2 Bass kernel for nn_CARLA_55430847922453.

Reference computation (per image, single step):
  x    = min(obs + (noise < hallucinogen), 1.0)          # hallucinate
  grid = 9 circular shifts of x (3x3 one-hot stencil)    # perceive
  h    = relu(w1 @ grid)                                 # 1x1 conv 9->32
  z    = w2 @ h + b2                                     # 1x1 conv 32->1
  out  = (sigmoid(z) > 0.5)  ==  (z > 0)                 # threshold
  out  = center crop 64x64 of the 256x256 grid

Key structural facts exploited:
  * Only the central 64x64 output crop is returned, so only a 66x66 input
    crop (rows/cols 95..160) feeds the result; no circular wrap occurs
    there.  The kernel reads just that window from DRAM.
  * obs in [0,1): min(obs + ind, 1) == max(obs, ind), ind = (noise < p).
  * sigmoid(z) > 0.5  <=>  z > 0, so no sigmoid is needed.
  * |w2_o| folds into w1 row o (relu(s*a) = s*relu(a), s>=0), leaving a
    +/-1 (sign) second layer.
  * bf16 matmul inputs are safe: decision margin |z| ~ 0.5 >> bf16 error.

Sharding: pure data parallel, 4 images per core across 8 cores.

Per-core dataflow:
  crops (2 DMAs) -> select (2 DVE ops) -> bf16 scratch in DRAM ->
  band tile [12, 4224] (1 contiguous DMA) -> pixel-form G [36, 4096]
  (shifted pixel-extract copies, split across both DMA rings and in
  column halves so the matmul loop starts after the first half) ->
  8x [ mm1 (K=36 blockdiag -> M=128) -> relu copy (ACT/DVE split) -> mm2
       (K=128 -> M=32 sign blockdiag, PE col-tile per chunk so z lands on
       16 PSUM partitions) ] ->
  fused bias+threshold DVE ops (the first z block overlaps the loop's
  second half) -> output DMAs split across both rings.

All weight/bias/threshold structures are built ON-CHIP (PE outer-product
broadcasts, a PE transpose against diag(|w2|), selector matrices and
blockdiag masks from gpsimd affine_select): DRAM parameter traffic is one
32-descriptor w1 load plus three 1-element loads.  This matters because
sub-32B HBM accesses cost ~50ns each (read-modify-write) and were the
dominant term in earlier revisions.

Written in raw Bass (explicit semaphores): the Tile layer's generated
sync exceeds this walrus build's per-instruction sync-slot limits (~2
waits per compute instruction).  Other ISA constraints honored here:
DMA access patterns balance to <= 3 dims, engine access patterns need
32-aligned partition starts, the PE stationary operand allows only one
free dim, and semaphore waits must target race-free (batch-sum) values.
"""

from contextlib import ExitStack

import numpy as np

import concourse.bass as bass
from concourse import mybir

F32 = mybir.dt.float32
BF16 = mybir.dt.bfloat16
I32 = mybir.dt.int32
AF = mybir.ActivationFunctionType
ALU = mybir.AluOpType

B, H, W = 32, 256, 256
N_CORES = 8
BL = B // N_CORES          # images per core
OUT = 64                   # output crop side
C0 = (H - OUT) // 2 - 1    # 95: first input row/col needed (1-pixel halo)
CS = OUT + 2               # 66: crop side with halo
HID = 32                   # hidden channels
NPIX = OUT * OUT           # pixels per image
CHUNK = 512
NCHUNK = NPIX // CHUNK     # 8
N_WARM2 = 0                # PE clock is throttle-capped; warmups useless
KDIM = 36                  # mm1 contraction: 3 c's x (4 images x 3 r's)
MDIM = BL * HID            # 128
BAND = OUT * CS            # 4224: one (image, r) band, 66-wide rows
N_WARM = 10                # PE warmup matmuls (HAM clock ungating)
WARM_N = 256
NHBUF = 4                  # h_bf rotation depth
NPBUF = 3                  # h_ps psum rotation depth


def _ap(t, offset, dims):
    """Raw access pattern on tensor/AP t's underlying tensor."""
    if not isinstance(t, bass.AP):
        t = t[:]
    return bass.AP(tensor=t.tensor, offset=t.offset + offset, ap=list(dims))


def build_program():
    nc = bass.Bass()

    obs = nc.declare_dram_parameter("obs", [BL, 1, H, W], F32, isOutput=False)
    noise = nc.declare_dram_parameter("noise", [BL, 1, H, W], F32, isOutput=False)
    w1 = nc.declare_dram_parameter("w1", [HID, 9], F32, isOutput=False)
    w2 = nc.declare_dram_parameter("w2", [1, HID], F32, isOutput=False)
    b2 = nc.declare_dram_parameter("b2", [1], F32, isOutput=False)
    hal = nc.declare_dram_parameter("hallucinogen", [1], F32, isOutput=False)
    act = nc.declare_dram_parameter("act", [BL, 1, OUT, OUT], F32, isOutput=True)

    scratch = nc.dram_tensor("scratch", [12, BAND], BF16)  # band-form
    w1_scr = nc.dram_tensor("w1_scr", [9, HID], BF16)  # [kappa, o]

    ctx = ExitStack()
    with ctx:
        ctx.enter_context(nc.allow_non_contiguous_dma(
            reason="tiny parameter-prep transposes/broadcasts"))
        names = [0]

        def sb(shape, dt):
            names[0] += 1
            return ctx.enter_context(
                nc.sbuf_tensor(f"sb{names[0]}", shape, dt))

        def ps(shape):
            names[0] += 1
            return ctx.enter_context(
                nc.psum_tensor(f"ps{names[0]}", shape, F32))

        # SBUF tensors
        obs_c = sb([CS, BL * CS], F32)
        noise_c = sb([CS, BL * CS], F32)
        ind = sb([CS, BL * CS], F32)
        x_bf = sb([CS, BL * CS], BF16)
        band = sb([12, BAND], BF16)
        gt = sb([KDIM, NPIX], BF16)
        junk_bf = sb([KDIM, WARM_N], BF16)
        ones_f32 = sb([1, 128], F32)
        ones_bf = sb([1, 128], BF16)
        ones32 = sb([32, 32], BF16)
        i32 = sb([32, 32], BF16)
        ones9 = sb([9, KDIM], BF16)
        sel9 = sb([9, KDIM], BF16)
        w1t_sb = sb([9, HID], BF16)
        w1_sb = sb([HID, 9], F32)
        w1_bf = sb([HID, 9], BF16)
        w2row = sb([1, HID], F32)
        w2abs_row = sb([1, HID], BF16)
        sgn_row = sb([1, HID], BF16)
        b2row = sb([1, 1], F32)
        p_row = sb([1, 1], F32)
        diag32 = sb([32, 32], BF16)
        w1rep = sb([KDIM, MDIM], BF16)
        t_band = [sb([KDIM, MDIM], BF16) for _ in range(3)]
        b_band = [sb([KDIM, MDIM], BF16) for _ in range(3)]
        w1blk = sb([KDIM, MDIM], BF16)
        diagmask = sb([MDIM, HID], BF16)
        w2blk = sb([MDIM, HID], BF16)
        h_bf = [sb([MDIM, CHUNK], BF16) for _ in range(NHBUF)]
        out_sb = sb([128, 2 * CHUNK], F32)

        # PSUM: warm(1 bank) + misc(1) + 3x h(3) + z(2) = 7 banks
        warm_ps = ps([1, WARM_N])
        misc_ps = ps([128, CHUNK])
        p_ps = misc_ps[0:CS, 0:1]
        b2_ps = misc_ps[0:128, 1:2]
        w2abs_ps = misc_ps[0:HID, 2:34]
        sgn_ps = misc_ps[0:MDIM, 34:66]
        w1x_ps = misc_ps[0:KDIM, 66:98]
        w1t_ps = misc_ps[0:9, 98:130]
        h_ps = [ps([MDIM, CHUNK]) for _ in range(NPBUF)]
        z_ps = ps([128, 2 * CHUNK])

        sA = ctx.enter_context(nc.semaphore("s_dma_sync"))   # sync-ring DMAs
        sB = ctx.enter_context(nc.semaphore("s_dma_act"))    # act-ring DMAs
        s_dve = ctx.enter_context(nc.semaphore("s_dve"))
        s_act = ctx.enter_context(nc.semaphore("s_act"))
        s_pool = ctx.enter_context(nc.semaphore("s_pool"))
        s_pe = ctx.enter_context(nc.semaphore("s_pe"))

        blk = ctx.enter_context(nc.Block())

        # relu engine split: even chunks on ACT, odd on DVE
        relu_on_act = [i % 2 == 0 for i in range(NCHUNK)]
        # --- engine-sem checkpoints (instruction counts per engine) ---
        DVE_XBF = 7      # junk, 4x ones, ind, x_bf
        DVE_W1REP = 14   # + diag32, w1t copy, 4x w1rep
        DVE_PREP = 22    # + 2 band adds, 6 diagmask/w2blk ops
        THRESH0 = DVE_PREP + 2 + 1   # 2 DVE relus (chunks 1,3) + thresh a=0
        THRESH = DVE_PREP + NCHUNK // 2 + 2   # all relus + both thresholds
        ACT_PREP = 2     # Abs, Sign
        PE_BASE = N_WARM + 6 + N_WARM2   # warmups + 6 prep mms + warmups2
        P_MMP = N_WARM + 1
        P_MMA = N_WARM + 3
        P_MMS = N_WARM + 4
        P_MMT = N_WARM + 5
        P_MMX = N_WARM + 6
        GH1_A = 6 * 16       # sA: crops, band writes 0-1, first-half c=0,1
        GH2_A = 8 * 16       # sA: + second-half c=0,1
        GH1_B = 6 * 16       # sB: params, band write 2, first-half c=2
        GH2_B = 7 * 16       # sB: + second-half c=2
        P_GATE = 2 * 16      # sB: p and b2 loads done
        WARM2_GATE = 4 * 16  # sA: band writes done (stable sem value)
        PARAMS = 4 * 16    # sB: w1, w2, b2, hal loads

        def dve_relu_count(i):
            n = DVE_PREP + sum(1 for j in range(i + 1) if not relu_on_act[j])
            return n + 1 if i >= 4 else n   # thresh a=0 interleaves after chunk 3

        def act_relu_count(i):
            return ACT_PREP + sum(1 for j in range(i + 1) if relu_on_act[j])

        def pe_mm1_count(i):
            # PE order: prep, mm1_0, mm1_1, [mm2_0, mm1_2], ..., mm2_6, mm2_7
            return PE_BASE + 1 + i if i < 2 else PE_BASE + 2 * i

        def pe_mm2_count(i):
            if i < NCHUNK - 2:
                return PE_BASE + 3 + 2 * i
            return PE_BASE + 2 * NCHUNK - (NCHUNK - 1 - i)

        @blk.sync
        def _(sync):
            cum = [0]

            def dma(out, in_):
                sync.dma_start(out=out, in_=in_).then_inc(sA, 16)
                cum[0] += 16

            crop_src = [[W, CS], [H * W, BL], [1, CS]]
            dma(obs_c[:], _ap(obs, C0 * W + C0, crop_src))
            dma(noise_c[:], _ap(noise, C0 * W + C0, crop_src))
            # band-form scratch: row 3g+r = crop_g rows r..r+64, 66-wide;
            # one write per r (r=0,1 here, r=2 on the ACT ring in parallel)
            sync.wait_ge(s_dve, DVE_XBF)
            for r in range(2):
                dma(_ap(scratch, r * BAND, [[CS, OUT], [3 * BAND, BL], [1, CS]]),
                    x_bf[r:r + OUT, :])
            # pixel-extract copies straight from DRAM scratch, in column
            # halves so the matmul loop starts after the first half
            for h in range(2):
                sync.wait_ge(sA, cum[0])
                if h == 0:
                    sync.wait_ge(sB, 5 * 16)   # band write r=2 done
                for c in range(2):
                    dma(gt[12 * c:12 * (c + 1), 2048 * h:2048 * (h + 1)],
                        _ap(scratch, c + 32 * h * CS,
                            [[BAND, 12], [CS, 32], [1, OUT]]))
            # output (first two column groups; other two on the ACT ring)
            sync.wait_ge(s_dve, THRESH0)
            for c in range(2):
                dma(_ap(act, c * CHUNK, [[NPIX, BL], [1, CHUNK]]),
                    out_sb[32 * c:32 * c + BL, 0:CHUNK])
            sync.wait_ge(s_dve, THRESH)
            for c in range(2):
                dma(_ap(act, c * CHUNK + 4 * CHUNK, [[NPIX, BL], [1, CHUNK]]),
                    out_sb[32 * c:32 * c + BL, CHUNK:])

        @blk.scalar
        def _(scalar):
            def dma(out, in_):
                scalar.dma_start(out=out, in_=in_).then_inc(sB, 16)

            dma(p_row[:], _ap(hal, 0, [[1, 1], [1, 1]]))
            dma(b2row[:], _ap(b2, 0, [[1, 1], [1, 1]]))
            scalar.wait_ge(sB, P_GATE)
            dma(w1_sb[:], w1[:, :])
            dma(w2row[:], w2[:, :])
            scalar.wait_ge(sB, PARAMS)
            nc.scalar.activation(w2abs_row[:], w2row[:], AF.Abs).then_inc(s_act, 1)
            nc.scalar.activation(sgn_row[:], w2row[:], AF.Sign).then_inc(s_act, 1)
            # band write r=2, then the c=2 pixel copies on this ring
            scalar.wait_ge(s_dve, DVE_XBF)
            dma(_ap(scratch, 2 * BAND, [[CS, OUT], [3 * BAND, BL], [1, CS]]),
                x_bf[2:2 + OUT, :])
            scalar.wait_ge(sB, 5 * 16)
            scalar.wait_ge(sA, 4 * 16)   # band writes r=0,1 done
            dma(gt[24:36, 0:2048],
                _ap(scratch, 2, [[BAND, 12], [CS, 32], [1, OUT]]))
            scalar.wait_ge(sB, GH1_B)
            dma(gt[24:36, 2048:4096],
                _ap(scratch, 2 + 32 * CS, [[BAND, 12], [CS, 32], [1, OUT]]))
            # ACT relu copies
            for i in range(NCHUNK):
                if relu_on_act[i]:
                    scalar.wait_ge(s_pe, pe_mm1_count(i))
                    if i >= NHBUF:  # WAR on h_bf slot
                        scalar.wait_ge(s_pe, pe_mm2_count(i - NHBUF))
                    nc.scalar.activation(h_bf[i % NHBUF][:], h_ps[i % NPBUF][:],
                                         AF.Relu).then_inc(s_act, 1)
            # output (remaining two column groups)
            scalar.wait_ge(s_dve, THRESH0)
            for c in range(2, 4):
                dma(_ap(act, c * CHUNK, [[NPIX, BL], [1, CHUNK]]),
                    out_sb[32 * c:32 * c + BL, 0:CHUNK])
            scalar.wait_ge(s_dve, THRESH)
            for c in range(2, 4):
                dma(_ap(act, c * CHUNK + 4 * CHUNK, [[NPIX, BL], [1, CHUNK]]),
                    out_sb[32 * c:32 * c + BL, CHUNK:])

        @blk.gpsimd
        def _(gpsimd):
            pn = [0]

            def inc(instr):
                instr.then_inc(s_pool, 1)
                pn[0] += 1

            # identity I32 via two is_ge selects (iota v = p - f)
            gpsimd.wait_ge(s_dve, 4)   # ones32
            inc(nc.gpsimd.affine_select(
                i32[:], ones32[:], pattern=[[-1, 32]], compare_op=ALU.is_ge,
                fill=0.0, base=0, channel_multiplier=1))
            gpsimd.wait_ge(s_pool, pn[0])
            inc(nc.gpsimd.affine_select(
                i32[:], i32[:], pattern=[[1, 32]], compare_op=ALU.is_ge,
                fill=0.0, base=0, channel_multiplier=-1))
            # selector sel9[kappa, 12c+3g+r] = [kappa == 3r+c]
            gpsimd.wait_ge(s_dve, 5)   # ones9
            selpat = [[1, 3], [0, BL], [3, 3]]
            inc(nc.gpsimd.affine_select(
                sel9[:], ones9[:], pattern=selpat, compare_op=ALU.is_ge,
                fill=0.0, base=0, channel_multiplier=-1))
            gpsimd.wait_ge(s_pool, pn[0])
            inc(nc.gpsimd.affine_select(
                sel9[:], sel9[:], pattern=[[-1, 3], [0, BL], [-3, 3]],
                compare_op=ALU.is_ge, fill=0.0, base=0, channel_multiplier=1))
            # blockdiag bands: keep w1rep where 0 <= k - 3g' - 12c <= 2
            gpsimd.wait_ge(s_dve, DVE_W1REP)
            pat = [[-3, BL], [0, HID]]
            for c in range(3):
                inc(nc.gpsimd.affine_select(
                    t_band[c][:], w1rep[:], pattern=pat, compare_op=ALU.is_ge,
                    fill=0.0, base=-12 * c, channel_multiplier=1))
                gpsimd.wait_ge(s_pool, pn[0])
                inc(nc.gpsimd.affine_select(
                    b_band[c][:], t_band[c][:],
                    pattern=[[3, BL], [0, HID]], compare_op=ALU.is_ge,
                    fill=0.0, base=12 * c + 2, channel_multiplier=-1))

        POOL_I32 = 2
        POOL_SEL = 4
        POOL_BANDS = 10

        @blk.vector
        def _(vector):
            dv = [0]

            def inc(instr):
                instr.then_inc(s_dve, 1)
                dv[0] += 1

            def selfwait():
                vector.wait_ge(s_dve, dv[0])

            # 1-4: constants
            inc(nc.vector.memset(junk_bf[:], 0.5))
            inc(nc.vector.memset(ones_f32[:], 1.0))
            inc(nc.vector.memset(ones_bf[:], 1.0))
            inc(nc.vector.memset(ones32[:], 1.0))
            inc(nc.vector.memset(ones9[:], 1.0))
            # hallucinate select first; the w1 copy follows so the scratch
            # chain is not gated on the slower 32-descriptor w1 load
            vector.wait_ge(sA, 32)
            vector.wait_ge(s_pe, P_MMP)
            inc(nc.vector.tensor_scalar(ind[:], noise_c[:], p_ps, None,
                                        ALU.is_lt))
            selfwait()
            inc(nc.vector.tensor_max(x_bf[:], ind[:], obs_c[:]))
            # w1 -> bf16
            vector.wait_ge(sB, PARAMS)
            inc(nc.vector.tensor_copy(w1_bf[:], w1_sb[:]))
            # 8: diag(|w2|)
            vector.wait_ge(s_pool, POOL_I32)
            vector.wait_ge(s_pe, P_MMA)
            inc(nc.vector.tensor_mul(diag32[:], i32[:], w2abs_ps))
            # w1t (transposed, |w2|-scaled) psum -> sbuf
            vector.wait_ge(s_pe, P_MMT)
            inc(nc.vector.tensor_copy(w1t_sb[:], w1t_ps))
            # w1rep from the PE-built staging
            vector.wait_ge(s_pe, P_MMX)
            for g in range(BL):
                inc(nc.vector.tensor_copy(w1rep[:, HID * g:HID * (g + 1)],
                                          w1x_ps))
            # 13-14: w1blk = sum of the 3 disjoint bands
            vector.wait_ge(s_pool, POOL_BANDS)
            inc(nc.vector.tensor_add(w1blk[:], b_band[0][:], b_band[1][:]))
            selfwait()
            inc(nc.vector.tensor_add(w1blk[:], w1blk[:], b_band[2][:]))
            # 15-20: w2blk = diagmask * sgn broadcast
            inc(nc.vector.memset(diagmask[:], 0.0))
            selfwait()
            for g in range(BL):
                inc(nc.vector.memset(diagmask[HID * g:HID * (g + 1), g:g + 1],
                                     1.0))
            vector.wait_ge(s_pe, P_MMS)
            selfwait()
            inc(nc.vector.tensor_mul(w2blk[:], diagmask[:], sgn_ps))
            # == DVE_PREP ops ==
            # DVE relu copies, with the a=0 threshold interleaved after
            # chunk 3 (its z block is complete then, overlapping chunks 4-7)
            for i in range(NCHUNK):
                if not relu_on_act[i]:
                    vector.wait_ge(s_pe, pe_mm1_count(i))
                    if i >= NHBUF:
                        vector.wait_ge(s_pe, pe_mm2_count(i - NHBUF))
                    inc(nc.vector.tensor_scalar_max(h_bf[i % NHBUF][:],
                                                    h_ps[i % NPBUF][:], 0.0))
                if i == 3:
                    vector.wait_ge(s_pe, pe_mm2_count(3))
                    inc(nc.vector.tensor_scalar(
                        out_sb[:, 0:CHUNK], z_ps[:, 0:CHUNK],
                        b2_ps, 0.0, ALU.add, ALU.is_gt))
            vector.wait_ge(s_pe, PE_BASE + 2 * NCHUNK)
            inc(nc.vector.tensor_scalar(out_sb[:, CHUNK:], z_ps[:, CHUNK:],
                                        b2_ps, 0.0, ALU.add, ALU.is_gt))

        @blk.tensor
        def _(tensor):
            # warmup (HAM): junk matmuls
            tensor.wait_ge(s_dve, 1)
            for _ in range(N_WARM):
                nc.tensor.matmul(warm_ps[:], lhsT=junk_bf[:, 0:1],
                                 rhs=junk_bf[:], start=True,
                                 stop=True).then_inc(s_pe, 1)
            # parameter broadcasts / transposes, all on-chip
            tensor.wait_ge(sB, P_GATE)
            tensor.wait_ge(s_dve, 4)
            nc.tensor.matmul(p_ps, lhsT=ones_f32[:, 0:CS], rhs=p_row[:],
                             start=True, stop=True).then_inc(s_pe, 1)
            nc.tensor.matmul(b2_ps, lhsT=ones_f32[:], rhs=b2row[:],
                             start=True, stop=True).then_inc(s_pe, 1)
            tensor.wait_ge(s_act, 2)
            nc.tensor.matmul(w2abs_ps, lhsT=w2abs_row[:],
                             rhs=ones_bf[:, 0:HID],
                             start=True, stop=True).then_inc(s_pe, 1)
            nc.tensor.matmul(sgn_ps, lhsT=ones_bf[:], rhs=sgn_row[:],
                             start=True, stop=True).then_inc(s_pe, 1)
            # w1t [9, 32] = w1_bf.T @ diag(|w2|)  (transpose + scale)
            tensor.wait_ge(s_dve, 9)   # diag32
            nc.tensor.matmul(w1t_ps, lhsT=w1_bf[:], rhs=diag32[:],
                             start=True, stop=True).then_inc(s_pe, 1)
            # staging [36, 32] = sel9.T @ w1t  (row replication/permutation)
            tensor.wait_ge(s_dve, 10)  # w1t_sb
            tensor.wait_ge(s_pool, POOL_SEL)
            nc.tensor.matmul(w1x_ps, lhsT=sel9[:], rhs=w1t_sb[:],
                             start=True, stop=True).then_inc(s_pe, 1)

            # second warmup batch: keeps the HAM busy-window alive while
            # the G copies finish, so the loop runs at 2.4 GHz
            tensor.wait_ge(sA, WARM2_GATE)
            for _ in range(N_WARM2):
                nc.tensor.matmul(warm_ps[:], lhsT=junk_bf[:, 0:1],
                                 rhs=junk_bf[:], start=True,
                                 stop=True).then_inc(s_pe, 1)

            tensor.wait_ge(sA, GH1_A)
            tensor.wait_ge(sB, GH1_B)
            tensor.wait_ge(s_dve, DVE_PREP)

            def mm1(i):
                if i == 4:   # second half of G needed from chunk 4 on
                    tensor.wait_ge(sA, GH2_A)
                    tensor.wait_ge(sB, GH2_B)
                if i >= NPBUF:  # WAR: h_ps[i%NPBUF] drained by its relu
                    j = i - NPBUF
                    if relu_on_act[j]:
                        tensor.wait_ge(s_act, act_relu_count(j))
                    else:
                        tensor.wait_ge(s_dve, dve_relu_count(j))
                nc.tensor.matmul(h_ps[i % NPBUF][:], lhsT=w1blk[:],
                                 rhs=gt[:, i * CHUNK:(i + 1) * CHUNK],
                                 start=True, stop=True).then_inc(s_pe, 1)

            def mm2(i):
                if relu_on_act[i]:
                    tensor.wait_ge(s_act, act_relu_count(i))
                else:
                    tensor.wait_ge(s_dve, dve_relu_count(i))
                c, a = i % 4, i // 4
                nc.tensor.matmul(
                    z_ps[32 * c:32 * (c + 1), CHUNK * a:CHUNK * a + CHUNK],
                    lhsT=w2blk[:], rhs=h_bf[i % NHBUF][:],
                    start=True, stop=True,
                    tile_position=(0, 32 * c)).then_inc(s_pe, 1)

            mm1(0)
            mm1(1)
            for i in range(NCHUNK - 2):
                mm2(i)
                mm1(i + 2)
            mm2(NCHUNK - 2)
            mm2(NCHUNK - 1)

    return nc


_CACHED_NC = None


def kernel(**inputs):
    global _CACHED_NC
    if _CACHED_NC is None:
        _CACHED_NC = build_program()
    nc = _CACHED_NC

    obs = np.ascontiguousarray(inputs["obs"], dtype=np.float32)
    noise = np.ascontiguousarray(inputs["noise"], dtype=np.float32)
    w1 = np.ascontiguousarray(inputs["w1"], dtype=np.float32)
    w2 = np.ascontiguousarray(inputs["w2"], dtype=np.float32)
    b2 = np.ascontiguousarray(inputs["b2"], dtype=np.float32)
    hal = np.ascontiguousarray(inputs["hallucinogen"], dtype=np.float32)

    in_maps = []
    for c in range(N_CORES):
        sl = slice(c * BL, (c + 1) * BL)
        in_maps.append({
            "obs": np.ascontiguousarray(obs[sl]),
            "noise": np.ascontiguousarray(noise[sl]),
            "w1": w1, "w2": w2, "b2": b2, "hallucinogen": hal,
        })

    from concourse.bass_utils import run_bass_kernel_spmd
    res = run_bass_kernel_spmd(nc, in_maps, list(range(N_CORES)))
    kernel.last_results = res
    out = np.concatenate([res.results[c]["act"] for c in range(N_CORES)], axis=0)
    return out.astype(np.float32)
